# revision 9
# baseline (speedup 1.0000x reference)
"""MoE layer (router + top-k dispatch + per-expert FFN + weighted combine)
on 8 Trainium2 NeuronCores.

Sharding strategy (expert-parallel, host-side dispatch + combine):
  - Core e owns expert e's weights (W1[e], W2[e], b1[e], b2[e]).
  - The host computes the router (x @ Wg -> softmax -> top-k) to decide
    WHICH tokens go to which core (the dispatch step of the sharding),
    gathers each expert's tokens, and ships them transposed ([D, C]
    token-minor) so both FFN GEMMs run with contraction on the partition
    axis and zero on-device transposes.
  - Device output is yT = (relu(W1^T x + b1))^T W2 + b2, transposed [O, C] —
    the device does ONLY the two dense GEMMs; the softmax combine weight
    probs[token, e] is applied on the host during the unshard scatter-add
    (the "weighted return" half of the expert-parallel all-to-all).
  - The host unshard step scatters-adds w * y into the [B, O] output
    (token indices are unique within one expert).

Compute is bf16 (fp32 PSUM accumulation); combine weights stay fp32.
"""

import numpy as np
import ml_dtypes
import bass_rust

import concourse.bass as bass
import concourse.mybir as mybir
import concourse.tile as tile
from concourse.bass_utils import run_bass_kernel_spmd

P = 128
N_CORES = 8
CHUNK = 512

def _normalize_sync_waits(nc):
    """The walrus build in this toolchain rejects >1 sync wait on a single
    instruction (setupSyncWait: "Too many sync wait commands"), while Tile's
    semaphore assignment freely emits several. Hoist all but one wait of each
    instruction onto same-engine NOPs placed immediately before it — the
    engine stream is in-order, so stalling at the NOPs is semantically
    identical to a multi-wait instruction."""
    count = 0
    for f in nc.m.functions:
        for bb in f.blocks:
            out = []
            changed = False
            for ins in bb.instructions:
                si = ins.sync_info
                if si is not None and len(si.on_wait) > 1:
                    waits = list(si.on_wait)
                    for w in waits[:-1]:
                        count += 1
                        out.append(
                            mybir.InstNoOp(
                                name=f"I-nw{count}",
                                ins=[],
                                outs=[],
                                engine=ins.engine,
                                sync_info=bass_rust.SyncInfo(
                                    on_wait=[w], on_update=[]
                                ),
                            )
                        )
                    ins.sync_info = bass_rust.SyncInfo(
                        on_wait=[waits[-1]], on_update=list(si.on_update)
                    )
                    changed = True
                out.append(ins)
            if changed:
                bb.instructions = out
    return nc


def _build_program(D, H, O, C, chunks):
    f32, bf16 = mybir.dt.float32, mybir.dt.bfloat16
    KD, MH, MO = D // P, H // P, O // P
    AF = mybir.ActivationFunctionType

    nc = bass.Bass()
    xT = nc.declare_dram_parameter("xT", [D, C], bf16, isOutput=False)
    w1 = nc.declare_dram_parameter("w1", [D, H], bf16, isOutput=False)
    w2 = nc.declare_dram_parameter("w2", [H, O], bf16, isOutput=False)
    b1p = nc.declare_dram_parameter("b1p", [P, MH], f32, isOutput=False)
    b2p = nc.declare_dram_parameter("b2p", [P, MO], f32, isOutput=False)
    yT = nc.declare_dram_parameter("yT", [O, C], f32, isOutput=True)

    with tile.TileContext(nc) as tc:
        with (
            tc.tile_pool(name="weights", bufs=1) as wpool,
            tc.tile_pool(name="xc", bufs=2) as xcpool,
            tc.tile_pool(name="h", bufs=1) as hpool,
            tc.tile_pool(name="ob", bufs=4) as outpool,
            tc.tile_pool(name="ps_h", bufs=4, space="PSUM") as ps_h,
            tc.tile_pool(name="ps_y", bufs=3, space="PSUM") as ps_y,
            tc.tile_pool(name="ps_w", bufs=1, space="PSUM") as ps_w,
        ):
            # Cold start: only gpsimd/SP/Activation queues can trigger DMAs,
            # and each trigger costs ~0.6-1us of queue time, while HBM gives
            # ~358 GB/s total — so the wire order is chosen so the bytes the
            # PE needs first (chunk-0 x + the leading w1 columns) land first.
            # GEMM1 group g reads w1[:, kd, g*128:(g+1)*128] and tile deps
            # are per-DMA-region, so w1 is cut into per-kd column slices A
            # [0:256) / B [256:1024) / C [1024:2048) (2D triggers issue in
            # ~0.7us vs ~9us of descriptor generation for the merged 3D
            # form); the trailing half is merged 3D — its trigger time hides
            # behind chunk-0 compute.
            xT_r = xT.rearrange("(kd p) c -> p kd c", p=P)
            w1_sb = wpool.tile([P, KD, H], bf16)
            w1_r = w1.rearrange("(kd p) h -> p kd h", p=P)
            w2_sb = wpool.tile([P, MH, O], bf16)
            w2_r = w2.rearrange("(kh p) o -> p kh o", p=P)
            b1_sb = wpool.tile([P, MH], f32)
            b2_sb = wpool.tile([P, MO], f32)
            xc0 = xcpool.tile([P, KD, CHUNK], bf16, tag="xc")
            HB = H // 4

            # PE clock warm-up: the DVFS p-state ramps to full speed after
            # ~3us of continuous PE activity, and the PE sits idle from the
            # post-barrier start (~6.3us) until the first data lands
            # (~10.5us).  Fill that dead window with throwaway matmuls on an
            # uninitialized SBUF tile so the real GEMM groups start at full
            # clock.
            warm_sb = wpool.tile([P, CHUNK], bf16)
            nc.vector.memset(warm_sb[:], 0.0)
            for _ in range(8):
                pw = ps_w.tile([P, CHUNK], f32, tag="pw")
                nc.tensor.matmul(
                    pw[:], warm_sb[:, :P], warm_sb[:], start=True, stop=True
                )

            qs = (nc.sync, nc.gpsimd, nc.scalar)
            N0 = chunks[0]
            nc.scalar.dma_start(b1_sb[:], b1p[:])
            for kd in range(KD):
                qs[kd % 3].dma_start(xc0[:, kd, :N0], xT_r[:, kd, :N0])
                qs[(kd + 1) % 3].dma_start(w1_sb[:, kd, :256], w1_r[:, kd, :256])
            nc.scalar.dma_start(b2_sb[:], b2p[:])
            for i, (lo, hi) in enumerate(
                [(256, 512), (512, 1024), (1024, 1536), (1536, 2048)]
            ):
                for kd in range(KD):
                    qs[(kd + i) % 3].dma_start(
                        w1_sb[:, kd, lo:hi], w1_r[:, kd, lo:hi]
                    )
            for hb in (2, 3):
                nc.sync.dma_start(
                    w1_sb[:, :, hb * HB : (hb + 1) * HB],
                    w1_r[:, :, hb * HB : (hb + 1) * HB],
                )

            offs = [sum(chunks[:i]) for i in range(len(chunks))]

            def emit_gemms(ci, xc):
                N, c0 = chunks[ci], offs[ci]
                # GEMM1: h^T = relu(W1^T @ x^T + b1), evicted to SBUF as bf16.
                # h is split into two half-tiles so the next chunk's GEMM1 can
                # start evicting into the first half as soon as this chunk's
                # GEMM2 has consumed it (tile deps are per-tile, not
                # per-region) — removes the chunk-boundary WAW bubble.
                hT_a = hpool.tile([P, MH // 2, CHUNK], bf16, tag="h_a")
                hT_b = hpool.tile([P, MH // 2, CHUNK], bf16, tag="h_b")

                def h_slice(kh, N=N, hT_a=hT_a, hT_b=hT_b):
                    t = hT_a if kh < MH // 2 else hT_b
                    return t[:, kh % (MH // 2), :N]

                for mh in range(MH):
                    ph = ps_h.tile([P, CHUNK], f32, tag="ph")
                    for kd in range(KD):
                        nc.tensor.matmul(
                            ph[:, :N],
                            w1_sb[:, kd, mh * P : (mh + 1) * P],
                            xc[:, kd, :N],
                            start=(kd == 0),
                            stop=(kd == KD - 1),
                        )
                    nc.scalar.activation(
                        h_slice(mh), ph[:, :N], AF.Relu, bias=b1_sb[:, mh : mh + 1]
                    )

                # GEMM2: y^T = W2^T @ h^T + b2, evicted straight to DRAM; the
                # softmax combine weight is applied host-side at unshard.
                for mo in range(MO):
                    py = ps_y.tile([P, CHUNK], f32, tag="py")
                    for kh in range(MH):
                        nc.tensor.matmul(
                            py[:, :N],
                            w2_sb[:, kh, mo * P : (mo + 1) * P],
                            h_slice(kh),
                            start=(kh == 0),
                            stop=(kh == MH - 1),
                        )
                    ob = outpool.tile([P, CHUNK], f32, tag="ob")
                    nc.scalar.activation(
                        ob[:, :N], py[:, :N], AF.Identity, bias=b2_sb[:, mo : mo + 1]
                    )
                    nc.sync.dma_start(yT[mo * P : (mo + 1) * P, c0 : c0 + N], ob[:, :N])

            for ci in range(len(chunks)):
                if ci + 1 < len(chunks):
                    N1, c1 = chunks[ci + 1], offs[ci + 1]
                    xc_next = xcpool.tile([P, KD, CHUNK], bf16, tag="xc")
                    nc.gpsimd.dma_start(
                        xc_next[:, :, :N1], xT_r[:, :, c1 : c1 + N1]
                    )
                if ci == 0:
                    # w2 (8.4 MB) is first read ~65us in; triggering it here —
                    # behind the chunk-1 prefetch — keeps its transfer off the
                    # wire while the latency-critical w1 slices stream.
                    for j in range(0, MH, MH // 2):
                        nc.gpsimd.dma_start(
                            w2_sb[:, j : j + MH // 2, :],
                            w2_r[:, j : j + MH // 2, :],
                        )
                emit_gemms(ci, xc0)
                xc0 = xc_next if ci + 1 < len(chunks) else None
    return _normalize_sync_waits(nc)


def kernel(**inputs):
    x = np.ascontiguousarray(np.asarray(inputs["x"], dtype=np.float32))
    Wg = np.ascontiguousarray(np.asarray(inputs["Wg"], dtype=np.float32))
    W1 = np.asarray(inputs["W1"], dtype=np.float32)
    b1 = np.asarray(inputs["b1"], dtype=np.float32)
    W2 = np.asarray(inputs["W2"], dtype=np.float32)
    b2 = np.asarray(inputs["b2"], dtype=np.float32)
    k = int(np.asarray(inputs["k"]))

    B, D = x.shape
    E = Wg.shape[1]
    H = W1.shape[2]
    O = W2.shape[2]
    assert E == N_CORES, f"expert-per-core layout expects E == 8, got {E}"

    # Host-side router: logits -> softmax probs (combine weights) and top-k
    # expert membership (softmax is monotonic, so top-k on logits == top-k
    # on probs).
    logits = x @ Wg
    m = logits.max(axis=1, keepdims=True)
    p = np.exp(logits - m)
    probs = p / p.sum(axis=1, keepdims=True)
    kth = np.partition(logits, E - k, axis=1)[:, E - k]  # k-th largest per token
    routed = logits >= kth[:, None]  # [B, E] membership mask
    idx_per_e = [np.nonzero(routed[:, e])[0] for e in range(E)]
    counts = [len(ix) for ix in idx_per_e]

    # Capacity: pad the largest expert's token count to a multiple of 8.
    # Split into <=512-token chunks; keep every chunk >=256 (below that the
    # fixed per-matmul issue/LDWEIGHTS cost stops amortizing) by borrowing
    # from the previous full chunk.
    C = max(CHUNK, -(-max(counts) // 8) * 8)
    chunks = [CHUNK] * (C // CHUNK)
    rem = C % CHUNK
    if rem:
        if rem < 256 and chunks:
            chunks[-1] -= 256 - rem
            rem = 256
        chunks.append(rem)

    nc = _build_program(D, H, O, C, chunks)

    in_maps = []
    for e in range(E):
        idx = idx_per_e[e]
        pad = np.zeros(C, dtype=np.int64)
        pad[: counts[e]] = idx
        xT_e = np.ascontiguousarray(x[pad].T.astype(ml_dtypes.bfloat16))
        in_maps.append(
            {
                "xT": xT_e,
                "w1": np.ascontiguousarray(W1[e].astype(ml_dtypes.bfloat16)),
                "w2": np.ascontiguousarray(W2[e].astype(ml_dtypes.bfloat16)),
                "b1p": np.ascontiguousarray(b1[e].reshape(H // P, P).T),
                "b2p": np.ascontiguousarray(b2[e].reshape(O // P, P).T),
            }
        )

    res = run_bass_kernel_spmd(nc, in_maps, core_ids=list(range(N_CORES)))
    globals()["_last_results"] = res

    out = np.zeros((B, O), dtype=np.float32)
    for e in range(E):
        cnt = counts[e]
        if cnt:
            idx = idx_per_e[e]
            yT_e = res.results[e]["yT"]
            out[idx] += probs[idx, e : e + 1] * yT_e[:, :cnt].T
    return out
